# revision 25
# baseline (speedup 1.0000x reference)
"""Megatron-style MoE layer (precomputed routing) on 8 Trainium2 NeuronCores.

Strategy: expert parallelism with F-split pairing for load balance. Experts
are paired (largest token count with smallest); the pair's two experts live
on a pair of cores, each core holding HALF of the ffn dimension F of both
experts (w1[:, half], w2[half, :] — 16 MB bf16, SBUF-resident). Both cores
of a pair process the SAME tokens (the union of both experts' dispatched
tokens) through their F-half and emit partial fc2 outputs; the host sums
the two partials. This makes the per-core PE load (count[a]+count[b])/2,
i.e. nearly perfectly balanced, instead of max_e count[e].

Per core, features-on-partition layout ([features, tokens]) so both matmuls
use the natural weight layout as lhsT with no on-chip transposes:

    y_part = coef * (gelu_tanh(x_seg @ w1h[seg]) @ w2h[seg])   seg in {b, a}

Weights are stored pre-blocked into [128, 128] matmul tiles so every weight
DMA moves 2 KB contiguous lines (256 B lines starve the opening cascade).

Device layouts (per core), P = 128 partitions, F2 = F/2 = 2048:
  xT   [P, 8, CB+CA]      bf16  x^T, h = ko*128 + p (seg b cols [0,CB), a after)
  w1   [P, 2, 16, 8, 128] bf16  [p, seg, fb, ko, j] = w1[e_seg][ko*128+p, fb*128+j]
  w2   [P, 2, 8, 16, 128] bf16  [p, seg, mh, kf, j] = w2[e_seg][kf*128+p, mh*128+j]
  coef [P, CB+CA]         bf16  per-token gate prob, replicated across partitions
  y    [P, 8, CB+CA]      bf16  partial y^T, hh = mh*128 + p
"""

import sys
import numpy as np
import ml_dtypes


def _ensure_axon_hooks():
    """bass_utils imports antenv.axon_hooks when BASS_TRACE is set; this
    image ships an antenv stub without it. Provide a working (or None)
    hook so tracing requests degrade gracefully instead of crashing."""
    try:
        import antenv.axon_hooks  # noqa: F401
        return
    except ImportError:
        pass
    import os
    import types

    mod = types.ModuleType("antenv.axon_hooks")
    state = [None]

    def set_axon_ntff_profile_hook(h):
        state[0] = h

    def get_axon_ntff_profile_hook():
        if state[0] is None:
            try:
                from trn_agent_boot.trn_boot import _ntff_profile_via_ctypes
                so = os.environ.get("PJRT_LIBRARY_PATH",
                                    "/opt/axon/libaxon_pjrt.so")
                if os.path.exists(so):
                    state[0] = _ntff_profile_via_ctypes(so)
            except Exception:
                pass
        return state[0]

    mod.set_axon_ntff_profile_hook = set_axon_ntff_profile_hook
    mod.get_axon_ntff_profile_hook = get_axon_ntff_profile_hook
    sys.modules["antenv.axon_hooks"] = mod
    try:
        import antenv
        antenv.axon_hooks = mod
    except ImportError:
        pass
    try:
        from concourse import bass_utils as _bu
        _orig = _bu.upload_artifacts

        def _safe_upload(tmpdir):
            try:
                return _orig(tmpdir)
            except Exception:
                return "local://" + tmpdir

        _bu.upload_artifacts = _safe_upload
    except Exception:
        pass


S, B, H = 1024, 8, 1024
T = S * B
E, K, F = 8, 2, 4096
F2 = F // 2
P = 128
NCORES = 8

_CACHE: dict[tuple, object] = {}

TRACE = False
LAST_RESULTS = None


def _tile_sizes(C: int) -> list[int]:
    """Split C into near-even tiles of at most 448 (descending) so none is
    LDWEIGHTS-bound. 448 rather than the 512 PSUM bank limit: full-bank
    (N=512) matmuls measure a steady +1.5% per-MM penalty on HW, while
    N<=448 matmuls stream at exactly N/2.4GHz + 2.5ns. Arbitrary (even
    non-multiple-of-32) sizes are fine for the PE/DMA."""
    assert C > 0
    nt = -(-C // 448)
    q, r = divmod(C, nt)
    return [q + 1] * r + [q] * (nt - r)


def _build(CB: int, CA: int):
    import concourse.bacc as bacc
    import concourse.mybir as mybir
    import concourse.tile as tile

    dt = mybir.dt
    AF = mybir.ActivationFunctionType

    nc = bacc.Bacc("TRN2", target_bir_lowering=False, debug=False,
                   num_devices=NCORES)

    C2 = CB + CA
    # x is packed per-tile-contiguous on the host (8*N consecutive elements
    # per partition per tile) so every x DMA moves multi-KB lines instead of
    # N-column slices with 2*N-byte lines
    xT_d = nc.dram_tensor("xT", [P, 8 * C2], dt.bfloat16, kind="ExternalInput").ap()
    w1_d = nc.dram_tensor("w1", [P, 2, 16, 8, 128], dt.bfloat16,
                          kind="ExternalInput").ap()
    w2_d = nc.dram_tensor("w2", [P, 2, 8, 16, 128], dt.bfloat16,
                          kind="ExternalInput").ap()
    cf_d = nc.dram_tensor("coef", [P, C2], dt.bfloat16, kind="ExternalInput").ap()
    y_d = nc.dram_tensor("y", [P, 8, C2], dt.bfloat16, kind="ExternalOutput").ap()

    # seg 0 = small expert (starts with the biggest tile: best compute/DMA
    # ratio during the opening ramp), seg 1 = big expert (ends with the
    # smallest tile: shortest drain)
    seg_tiles = [_tile_sizes(CB), _tile_sizes(CA)]
    seg_off = [0, CB]

    with tile.TileContext(nc) as tc:
        with (
            tc.tile_pool(name="wpool", bufs=1) as wpool,
            tc.tile_pool(name="xpool", bufs=2) as xpool,
            tc.tile_pool(name="hpool", bufs=1) as hpool,
            tc.tile_pool(name="opool", bufs=2) as opool,
            tc.tile_pool(name="opool8", bufs=2) as opool8,
            tc.tile_pool(name="ps1", bufs=3, space="PSUM") as ps1,
            tc.tile_pool(name="ps2", bufs=3, space="PSUM") as ps2,
            tc.tile_pool(name="psw", bufs=1, space="PSUM") as psw,
        ):
            w1_sb = wpool.tile([P, 2, 16, 8, 128], dt.bfloat16, tag="w1")
            w2_sb = wpool.tile([P, 2, 8, 16, 128], dt.bfloat16, tag="w2")

            # PE warmup: the HAM clock gate holds the PE at 1.2 GHz until it
            # has been busy ~3.4us. Scratch matmuls (no DMA dependencies) run
            # while the opening DMAs are in flight, so the PE is already at
            # 2.4 GHz when real compute starts.
            # Init the scratch on the vector engine (earliest BB entry of the
            # idle engines — gpsimd took ~1.5us longer to come up).
            warm = wpool.tile([P, 128], dt.bfloat16, tag="warm")
            nc.vector.memset(warm[:, :], 0)
            wp = psw.tile([P, 128], dt.float32, tag="wp")
            NWARM = 38
            for i in range(NWARM):
                nc.tensor.matmul(wp[:, :], warm[:, :], warm[:, :],
                                 start=(i == 0), stop=(i == NWARM - 1))

            # Each dma_start costs ~0.5us of issue time on its engine, and an
            # engine's DMAs complete in issue order. Issue the first x tile
            # from the scalar engine (the OTHER hardware DGE queue besides
            # sync; its gelu work only starts much later) in parallel with
            # the fc1 weight blocks on sync, then stream the remaining seg-0
            # fc1 weight blocks just ahead of mf-group consumption.
            # (gpsimd's DMA path measured ~100us slower — never use it.)
            N0 = seg_tiles[0][0]
            xt0 = xpool.tile([P, 8 * 512], dt.bfloat16, tag="x")
            cf0 = xpool.tile([P, 512], dt.bfloat16, tag="cf")
            nc.scalar.dma_start(xt0[:, 0:4 * N0], xT_d[:, 0:4 * N0])
            nc.scalar.dma_start(xt0[:, 4 * N0:8 * N0], xT_d[:, 4 * N0:8 * N0])
            for fb in range(16):
                nc.sync.dma_start(w1_sb[:, 0, fb, :, :], w1_d[:, 0, fb, :, :])
            nc.sync.dma_start(cf0[:, :N0], cf_d[:, 0:N0])
            # w2 seg 0 by output block pairs: mh=0 is needed first (fc2 of
            # tile 0)
            for i in range(4):
                nc.sync.dma_start(w2_sb[:, 0, 2 * i:2 * i + 2, :, :],
                                  w2_d[:, 0, 2 * i:2 * i + 2, :, :])

            w_late_issued = [False, False]  # w1 seg 1, w2 seg 1

            for s in range(2):
                for ti, N in enumerate(seg_tiles[s]):
                    t0 = seg_off[s] + sum(seg_tiles[s][:ti])
                    if s == 0 and ti == 0:
                        xt, cf = xt0, cf0
                    else:
                        xt = xpool.tile([P, 8 * 512], dt.bfloat16, tag="x")
                        nc.sync.dma_start(xt[:, :8 * N],
                                          xT_d[:, 8 * t0:8 * t0 + 8 * N])
                        cf = xpool.tile([P, 512], dt.bfloat16, tag="cf")
                        nc.sync.dma_start(cf[:, :N], cf_d[:, t0:t0 + N])

                    h = hpool.tile([P, 16, 512], dt.bfloat16, tag="h")
                    for mf in range(16):
                        p1 = ps1.tile([P, 512], dt.float32, tag="p1")
                        for ko in range(8):
                            nc.tensor.matmul(
                                p1[:, :N],
                                w1_sb[:, s, mf, ko, :],
                                xt[:, ko * N:(ko + 1) * N],
                                start=(ko == 0), stop=(ko == 7),
                            )
                        nc.scalar.activation(h[:, mf, :N], p1[:, :N],
                                             AF.Gelu_apprx_tanh)

                    # stage seg-1 weights behind the early seg-0 compute,
                    # off tile 0's critical DMA path
                    if s == 0 and not w_late_issued[0] and (
                            ti >= 1 or ti == len(seg_tiles[0]) - 1):
                        for i in range(2):
                            nc.sync.dma_start(
                                w1_sb[:, 1, 8 * i:8 * i + 8, :, :],
                                w1_d[:, 1, 8 * i:8 * i + 8, :, :])
                        w_late_issued[0] = True
                    elif s == 0 and not w_late_issued[1] and (
                            ti >= 2 or ti == len(seg_tiles[0]) - 1):
                        for i in range(2):
                            nc.sync.dma_start(
                                w2_sb[:, 1, 4 * i:4 * i + 4, :, :],
                                w2_d[:, 1, 4 * i:4 * i + 4, :, :])
                        w_late_issued[1] = True

                    last_tile = (s == 1 and ti == len(seg_tiles[1]) - 1)
                    if not last_tile:
                        # batch the 8 output blocks into one strided DMA:
                        # 7 fewer issue ops and completion events per tile
                        # (DMA event traffic measurably inflates PE issue)
                        ot8 = opool8.tile([P, 8, 512], dt.bfloat16, tag="o8")
                        for mh in range(8):
                            p2 = ps2.tile([P, 512], dt.float32, tag="p2")
                            for kf in range(16):
                                nc.tensor.matmul(
                                    p2[:, :N],
                                    w2_sb[:, s, mh, kf, :],
                                    h[:, kf, :N],
                                    start=(kf == 0), stop=(kf == 15),
                                )
                            nc.vector.tensor_mul(ot8[:, mh, :N], p2[:, :N],
                                                 cf[:, :N])
                        nc.sync.dma_start(y_d[:, :, t0:t0 + N],
                                          ot8[:, :, :N])
                        continue
                    for mh in range(8):
                        if last_tile and mh == 7:
                            # split the final group asymmetrically so the
                            # last multiply+DMA covers a short chunk that
                            # overlaps the first chunk's output; issue the
                            # final DMA from the idle scalar queue so both
                            # issue in parallel. (bf16 FWL keeps even narrow
                            # chunks streaming at N/2.4, not LDW-bound.)
                            nh = max(32, (3 * N // 4) // 32 * 32)
                            for ci, (c0, cn) in enumerate(
                                    ((0, nh), (nh, N - nh))):
                                p2 = ps2.tile([P, 512], dt.float32, tag="p2")
                                for kf in range(16):
                                    nc.tensor.matmul(
                                        p2[:, :cn],
                                        w2_sb[:, s, mh, kf, :],
                                        h[:, kf, c0:c0 + cn],
                                        start=(kf == 0), stop=(kf == 15),
                                    )
                                ot = opool.tile([P, 512], dt.bfloat16, tag="o")
                                nc.vector.tensor_mul(ot[:, :cn], p2[:, :cn],
                                                     cf[:, c0:c0 + cn])
                                eng = nc.scalar if ci == 1 else nc.sync
                                eng.dma_start(
                                    y_d[:, mh, t0 + c0:t0 + c0 + cn],
                                    ot[:, :cn])
                            continue
                        p2 = ps2.tile([P, 512], dt.float32, tag="p2")
                        for kf in range(16):
                            nc.tensor.matmul(
                                p2[:, :N],
                                w2_sb[:, s, mh, kf, :],
                                h[:, kf, :N],
                                start=(kf == 0), stop=(kf == 15),
                            )
                        ot = opool.tile([P, 512], dt.bfloat16, tag="o")
                        nc.vector.tensor_mul(ot[:, :N], p2[:, :N], cf[:, :N])
                        nc.sync.dma_start(y_d[:, mh, t0:t0 + N], ot[:, :N])

                # safety for degenerate shapes: make sure seg-1 weights are
                # staged before seg 1 runs
                if s == 0:
                    if not w_late_issued[0]:
                        nc.sync.dma_start(w1_sb[:, 1], w1_d[:, 1])
                        w_late_issued[0] = True
                    if not w_late_issued[1]:
                        nc.sync.dma_start(w2_sb[:, 1], w2_d[:, 1])
                        w_late_issued[1] = True

    nc.compile()
    return nc


def kernel(hidden_states, gate_weight, choosed_experts, w1, w2):
    global LAST_RESULTS
    _ensure_axon_hooks()
    from concourse import bass_utils

    x = np.asarray(hidden_states, dtype=np.float32).reshape(T, H)
    gw = np.asarray(gate_weight, dtype=np.float32)
    ce = np.asarray(choosed_experts).astype(np.int64)
    w1 = np.asarray(w1, dtype=np.float32)
    w2 = np.asarray(w2, dtype=np.float32)

    # routing: stable sort of (token, k) pairs by expert
    flat = ce.reshape(-1)
    order = np.argsort(flat, kind="stable")
    counts = np.bincount(flat, minlength=E).astype(np.int64)
    starts = np.zeros(E + 1, dtype=np.int64)
    starts[1:] = np.cumsum(counts)

    # pair largest with smallest so each pair's total is near T*K/4
    desc = np.argsort(-counts, kind="stable")
    pairs = [(int(desc[p]), int(desc[7 - p])) for p in range(4)]
    # pad only to 8 elements (16 B lines) — token-slot compute scales with
    # CA+CB, so every padded slot is wasted matmul columns
    pad8 = lambda n: max(32, -(-int(n) // 8) * 8)
    CA = pad8(counts[desc[0]])
    CB = pad8(counts[desc[4]])

    nc = _CACHE.get((CB, CA))
    if nc is None:
        nc = _build(CB, CA)
        _CACHE[(CB, CA)] = nc

    bf16 = ml_dtypes.bfloat16

    # per-expert gathered activations / coefs (shared by both cores of a pair)
    def gather(e, C):
        p = order[starts[e]:starts[e + 1]]
        t_idx = p // K
        k_idx = p % K
        n_e = len(p)
        xT = np.zeros((H, C), dtype=bf16)
        xT[:, :n_e] = x[t_idx].T
        xT = xT.reshape(8, P, C).transpose(1, 0, 2)
        coef = np.zeros((C,), dtype=np.float32)
        coef[:n_e] = gw[t_idx, k_idx]
        return xT, coef

    # per-tile-contiguous flat x layout, matching the kernel's tiling
    tiles_flat = []
    t0 = 0
    for Cseg in (CB, CA):
        for N in _tile_sizes(Cseg):
            tiles_flat.append((t0, N))
            t0 += N

    in_maps = [None] * NCORES
    for pi, (ea, eb) in enumerate(pairs):
        xa, ca_ = gather(ea, CA)
        xb, cb_ = gather(eb, CB)
        # seg 0 = small expert (b), seg 1 = big expert (a)
        xcat = np.concatenate([xb, xa], axis=2)
        xT = np.empty((P, 8 * (CB + CA)), dtype=bf16)
        o = 0
        for (tt0, N) in tiles_flat:
            xT[:, o:o + 8 * N] = xcat[:, :, tt0:tt0 + N].reshape(P, 8 * N)
            o += 8 * N
        coef = np.concatenate([cb_, ca_]).astype(bf16)
        coef = np.ascontiguousarray(
            np.broadcast_to(coef[None, :], (P, CB + CA)))
        for half in range(2):
            sl = slice(half * F2, (half + 1) * F2)
            # w1 blocked [p, seg, fb, ko, j]; w2 blocked [p, seg, mh, kf, j]
            w1h = np.stack([
                w1[e][:, sl].astype(bf16).reshape(8, P, 16, 128)
                .transpose(1, 2, 0, 3)
                for e in (eb, ea)], axis=1)
            w2h = np.stack([
                w2[e][sl, :].astype(bf16).reshape(16, P, 8, 128)
                .transpose(1, 2, 0, 3)
                for e in (eb, ea)], axis=1)
            in_maps[2 * pi + half] = {
                "xT": xT, "w1": np.ascontiguousarray(w1h),
                "w2": np.ascontiguousarray(w2h), "coef": coef,
            }

    res = bass_utils.run_bass_kernel_spmd(nc, in_maps, list(range(NCORES)),
                                          trace=TRACE)
    LAST_RESULTS = res

    # combine: sum the two F-half partials per pair, split back per expert,
    # then sum the K contributions per token
    ys = [None] * E
    for pi, (ea, eb) in enumerate(pairs):
        ysum = (res.results[2 * pi]["y"].astype(np.float32)
                + res.results[2 * pi + 1]["y"].astype(np.float32))
        yT = ysum.transpose(1, 0, 2).reshape(H, CB + CA)
        ys[eb] = yT[:, :counts[eb]].T
        ys[ea] = yT[:, CB:CB + counts[ea]].T
    all_pairs = np.concatenate([ys[e] for e in range(E)], axis=0)
    out_pairs = np.empty((T * K, H), dtype=np.float32)
    out_pairs[order] = all_pairs
    return out_pairs.reshape(T, K, H).sum(axis=1)


# revision 30
# speedup vs baseline: 1.0679x; 1.0679x over previous
"""Megatron-style MoE layer (precomputed routing) on 8 Trainium2 NeuronCores.

Strategy: expert parallelism with F-split pairing for load balance. Experts
are paired (largest token count with smallest); the pair's two experts live
on a pair of cores, each core holding HALF of the ffn dimension F of both
experts (w1[:, half], w2[half, :] — 16 MB bf16, SBUF-resident). Both cores
of a pair process the SAME tokens (the union of both experts' dispatched
tokens) through their F-half and emit partial fc2 outputs; the host sums
the two partials. This makes the per-core PE load (count[a]+count[b])/2,
i.e. nearly perfectly balanced, instead of max_e count[e].

Per core, features-on-partition layout ([features, tokens]) so both matmuls
use the natural weight layout as lhsT with no on-chip transposes:

    y_part = coef * (gelu_tanh(x_seg @ w1h[seg]) @ w2h[seg])   seg in {b, a}

Weights are stored pre-blocked into [128, 128] matmul tiles so every weight
DMA moves 2 KB contiguous lines (256 B lines starve the opening cascade).

Device layouts (per core), P = 128 partitions, F2 = F/2 = 2048:
  xT   [P, 8, CB+CA]      bf16  x^T, h = ko*128 + p (seg b cols [0,CB), a after)
  w1   [P, 2, 16, 8, 128] bf16  [p, seg, fb, ko, j] = w1[e_seg][ko*128+p, fb*128+j]
  w2   [P, 2, 8, 16, 128] bf16  [p, seg, mh, kf, j] = w2[e_seg][kf*128+p, mh*128+j]
  coef [P, CB+CA]         bf16  per-token gate prob, replicated across partitions
  y    [P, 8, CB+CA]      bf16  partial y^T, hh = mh*128 + p
"""

import sys
import numpy as np
import ml_dtypes


def _ensure_axon_hooks():
    """bass_utils imports antenv.axon_hooks when BASS_TRACE is set; this
    image ships an antenv stub without it. Provide a working (or None)
    hook so tracing requests degrade gracefully instead of crashing."""
    try:
        import antenv.axon_hooks  # noqa: F401
        return
    except ImportError:
        pass
    import os
    import types

    mod = types.ModuleType("antenv.axon_hooks")
    state = [None]

    def set_axon_ntff_profile_hook(h):
        state[0] = h

    def get_axon_ntff_profile_hook():
        if state[0] is None:
            try:
                from trn_agent_boot.trn_boot import _ntff_profile_via_ctypes
                so = os.environ.get("PJRT_LIBRARY_PATH",
                                    "/opt/axon/libaxon_pjrt.so")
                if os.path.exists(so):
                    state[0] = _ntff_profile_via_ctypes(so)
            except Exception:
                pass
        return state[0]

    mod.set_axon_ntff_profile_hook = set_axon_ntff_profile_hook
    mod.get_axon_ntff_profile_hook = get_axon_ntff_profile_hook
    sys.modules["antenv.axon_hooks"] = mod
    try:
        import antenv
        antenv.axon_hooks = mod
    except ImportError:
        pass
    try:
        from concourse import bass_utils as _bu
        _orig = _bu.upload_artifacts

        def _safe_upload(tmpdir):
            try:
                return _orig(tmpdir)
            except Exception:
                return "local://" + tmpdir

        _bu.upload_artifacts = _safe_upload
    except Exception:
        pass


S, B, H = 1024, 8, 1024
T = S * B
E, K, F = 8, 2, 4096
F2 = F // 2
P = 128
NCORES = 8

_CACHE: dict[tuple, object] = {}

TRACE = False
LAST_RESULTS = None


def _tile_sizes(C: int) -> list[int]:
    """Split C into near-even tiles of at most 448 (descending) so none is
    LDWEIGHTS-bound. 448 rather than the 512 PSUM bank limit: full-bank
    (N=512) matmuls measure a steady +1.5% per-MM penalty on HW, while
    N<=448 matmuls stream at exactly N/2.4GHz + 2.5ns. Arbitrary (even
    non-multiple-of-32) sizes are fine for the PE/DMA."""
    assert C > 0
    nt = -(-C // 448)
    q, r = divmod(C, nt)
    return [q + 1] * r + [q] * (nt - r)


def _build(CB: int, CA: int):
    import concourse.bacc as bacc
    import concourse.mybir as mybir
    import concourse.tile as tile

    dt = mybir.dt
    AF = mybir.ActivationFunctionType

    nc = bacc.Bacc("TRN2", target_bir_lowering=False, debug=False,
                   num_devices=NCORES)

    C2 = CB + CA
    # x is packed per-tile-contiguous on the host (8*N consecutive elements
    # per partition per tile) so every x DMA moves multi-KB lines instead of
    # N-column slices with 2*N-byte lines
    xT_d = nc.dram_tensor("xT", [P, 8 * C2], dt.bfloat16, kind="ExternalInput").ap()
    w1_d = nc.dram_tensor("w1", [P, 2, 16, 8, 128], dt.bfloat16,
                          kind="ExternalInput").ap()
    w2_d = nc.dram_tensor("w2", [P, 2, 8, 16, 128], dt.bfloat16,
                          kind="ExternalInput").ap()
    cf_d = nc.dram_tensor("coef", [P, C2], dt.bfloat16, kind="ExternalInput").ap()
    y_d = nc.dram_tensor("y", [P, 8, C2], dt.bfloat16, kind="ExternalOutput").ap()

    # seg 0 = small expert (starts with the biggest tile: best compute/DMA
    # ratio during the opening ramp), seg 1 = big expert (ends with the
    # smallest tile: shortest drain)
    seg_tiles = [_tile_sizes(CB), _tile_sizes(CA)]
    seg_off = [0, CB]

    with tile.TileContext(nc) as tc:
        with (
            tc.tile_pool(name="wpool", bufs=1) as wpool,
            tc.tile_pool(name="xpool", bufs=2) as xpool,
            tc.tile_pool(name="hpool", bufs=1) as hpool,
            tc.tile_pool(name="opool", bufs=2) as opool,
            tc.tile_pool(name="opool8", bufs=2) as opool8,
            tc.tile_pool(name="ps1", bufs=3, space="PSUM") as ps1,
            tc.tile_pool(name="ps2", bufs=3, space="PSUM") as ps2,
            tc.tile_pool(name="psw", bufs=1, space="PSUM") as psw,
        ):
            w1_sb = wpool.tile([P, 2, 16, 8, 128], dt.bfloat16, tag="w1")
            w2_sb = wpool.tile([P, 2, 8, 16, 128], dt.bfloat16, tag="w2")

            # PE warmup: the HAM clock gate holds the PE at 1.2 GHz until it
            # has been busy ~3.4us. Scratch matmuls (no DMA dependencies) run
            # while the opening DMAs are in flight, so the PE is already at
            # 2.4 GHz when real compute starts.
            # Init the scratch on the vector engine (earliest BB entry of the
            # idle engines — gpsimd took ~1.5us longer to come up).
            warm = wpool.tile([P, 128], dt.bfloat16, tag="warm")
            nc.vector.memset(warm[:, :], 0)
            wp = psw.tile([P, 128], dt.float32, tag="wp")
            NWARM = 34
            for i in range(NWARM):
                nc.tensor.matmul(wp[:, :], warm[:, :], warm[:, :],
                                 start=(i == 0), stop=(i == NWARM - 1))

            # Each dma_start costs ~0.5us of issue time on its engine, and an
            # engine's DMAs complete in issue order. Issue the first x tile
            # from the scalar engine (the OTHER hardware DGE queue besides
            # sync; its gelu work only starts much later) in parallel with
            # the fc1 weight blocks on sync, then stream the remaining seg-0
            # fc1 weight blocks just ahead of mf-group consumption.
            # (gpsimd's DMA path measured ~100us slower — never use it.)
            N0 = seg_tiles[0][0]
            xt0 = xpool.tile([P, 8 * 512], dt.bfloat16, tag="x")
            cf0 = xpool.tile([P, 512], dt.bfloat16, tag="cf")
            nc.scalar.dma_start(xt0[:, 0:4 * N0], xT_d[:, 0:4 * N0])
            nc.scalar.dma_start(xt0[:, 4 * N0:8 * N0], xT_d[:, 4 * N0:8 * N0])
            for fb in range(16):
                nc.sync.dma_start(w1_sb[:, 0, fb, :, :], w1_d[:, 0, fb, :, :])
            nc.sync.dma_start(cf0[:, :N0], cf_d[:, 0:N0])
            # w2 seg 0 by output block pairs: mh=0 is needed first (fc2 of
            # tile 0)
            for i in range(4):
                nc.sync.dma_start(w2_sb[:, 0, 2 * i:2 * i + 2, :, :],
                                  w2_d[:, 0, 2 * i:2 * i + 2, :, :])

            w_late_issued = [False, False]  # w1 seg 1, w2 seg 1

            for s in range(2):
                for ti, N in enumerate(seg_tiles[s]):
                    t0 = seg_off[s] + sum(seg_tiles[s][:ti])
                    if s == 0 and ti == 0:
                        xt, cf = xt0, cf0
                    else:
                        xt = xpool.tile([P, 8 * 512], dt.bfloat16, tag="x")
                        nc.sync.dma_start(xt[:, :8 * N],
                                          xT_d[:, 8 * t0:8 * t0 + 8 * N])
                        cf = xpool.tile([P, 512], dt.bfloat16, tag="cf")
                        nc.sync.dma_start(cf[:, :N], cf_d[:, t0:t0 + N])

                    h = hpool.tile([P, 16, 512], dt.bfloat16, tag="h")
                    for mf in range(16):
                        p1 = ps1.tile([P, 512], dt.float32, tag="p1")
                        for ko in range(8):
                            nc.tensor.matmul(
                                p1[:, :N],
                                w1_sb[:, s, mf, ko, :],
                                xt[:, ko * N:(ko + 1) * N],
                                start=(ko == 0), stop=(ko == 7),
                            )
                        nc.scalar.activation(h[:, mf, :N], p1[:, :N],
                                             AF.Gelu_apprx_tanh)

                    # stage seg-1 weights behind the early seg-0 compute,
                    # off tile 0's critical DMA path
                    if s == 0 and not w_late_issued[0] and (
                            ti >= 1 or ti == len(seg_tiles[0]) - 1):
                        for i in range(2):
                            nc.sync.dma_start(
                                w1_sb[:, 1, 8 * i:8 * i + 8, :, :],
                                w1_d[:, 1, 8 * i:8 * i + 8, :, :])
                        w_late_issued[0] = True
                    elif s == 0 and not w_late_issued[1] and (
                            ti >= 2 or ti == len(seg_tiles[0]) - 1):
                        for i in range(2):
                            nc.sync.dma_start(
                                w2_sb[:, 1, 4 * i:4 * i + 4, :, :],
                                w2_d[:, 1, 4 * i:4 * i + 4, :, :])
                        w_late_issued[1] = True

                    last_tile = (s == 1 and ti == len(seg_tiles[1]) - 1)
                    if not last_tile:
                        # batch the 8 output blocks into one strided DMA:
                        # 7 fewer issue ops and completion events per tile
                        # (DMA event traffic measurably inflates PE issue)
                        ot8 = opool8.tile([P, 8, 512], dt.bfloat16, tag="o8")
                        for mh in range(8):
                            p2 = ps2.tile([P, 512], dt.float32, tag="p2")
                            for kf in range(16):
                                nc.tensor.matmul(
                                    p2[:, :N],
                                    w2_sb[:, s, mh, kf, :],
                                    h[:, kf, :N],
                                    start=(kf == 0), stop=(kf == 15),
                                )
                            nc.vector.tensor_mul(ot8[:, mh, :N], p2[:, :N],
                                                 cf[:, :N])
                        nc.sync.dma_start(y_d[:, :, t0:t0 + N],
                                          ot8[:, :, :N])
                        continue
                    for mh in range(8):
                        if last_tile and mh == 7:
                            # split the final group in two so the last
                            # multiply+DMA overlaps the second half's matmuls
                            nh = ((N // 2) + 31) // 32 * 32
                            for (c0, cn) in ((0, nh), (nh, N - nh)):
                                p2 = ps2.tile([P, 512], dt.float32, tag="p2")
                                for kf in range(16):
                                    nc.tensor.matmul(
                                        p2[:, :cn],
                                        w2_sb[:, s, mh, kf, :],
                                        h[:, kf, c0:c0 + cn],
                                        start=(kf == 0), stop=(kf == 15),
                                    )
                                ot = opool.tile([P, 512], dt.bfloat16, tag="o")
                                nc.vector.tensor_mul(ot[:, :cn], p2[:, :cn],
                                                     cf[:, c0:c0 + cn])
                                nc.sync.dma_start(
                                    y_d[:, mh, t0 + c0:t0 + c0 + cn],
                                    ot[:, :cn])
                            continue
                        p2 = ps2.tile([P, 512], dt.float32, tag="p2")
                        for kf in range(16):
                            nc.tensor.matmul(
                                p2[:, :N],
                                w2_sb[:, s, mh, kf, :],
                                h[:, kf, :N],
                                start=(kf == 0), stop=(kf == 15),
                            )
                        ot = opool.tile([P, 512], dt.bfloat16, tag="o")
                        nc.vector.tensor_mul(ot[:, :N], p2[:, :N], cf[:, :N])
                        nc.sync.dma_start(y_d[:, mh, t0:t0 + N], ot[:, :N])

                # safety for degenerate shapes: make sure seg-1 weights are
                # staged before seg 1 runs
                if s == 0:
                    if not w_late_issued[0]:
                        nc.sync.dma_start(w1_sb[:, 1], w1_d[:, 1])
                        w_late_issued[0] = True
                    if not w_late_issued[1]:
                        nc.sync.dma_start(w2_sb[:, 1], w2_d[:, 1])
                        w_late_issued[1] = True

    nc.compile()
    return nc


def kernel(hidden_states, gate_weight, choosed_experts, w1, w2):
    global LAST_RESULTS
    _ensure_axon_hooks()
    from concourse import bass_utils

    x = np.asarray(hidden_states, dtype=np.float32).reshape(T, H)
    gw = np.asarray(gate_weight, dtype=np.float32)
    ce = np.asarray(choosed_experts).astype(np.int64)
    w1 = np.asarray(w1, dtype=np.float32)
    w2 = np.asarray(w2, dtype=np.float32)

    # routing: stable sort of (token, k) pairs by expert, then MERGE a
    # token's two picks when both hit the same expert (the reference sums
    # the gate weights in that case — identical math, ~1/8 of tokens, so
    # ~6% fewer rows to compute)
    flat = ce.reshape(-1)
    order = np.argsort(flat, kind="stable")
    raw_counts = np.bincount(flat, minlength=E).astype(np.int64)
    starts = np.zeros(E + 1, dtype=np.int64)
    starts[1:] = np.cumsum(raw_counts)

    tok_lists = []
    for e in range(E):
        p = order[starts[e]:starts[e + 1]]
        t_idx = p // K
        k_idx = p % K
        ut, inv = np.unique(t_idx, return_inverse=True)
        cf = np.zeros(len(ut), dtype=np.float32)
        np.add.at(cf, inv, gw[t_idx, k_idx])
        tok_lists.append((ut, cf))
    counts = np.array([len(ut) for ut, _ in tok_lists], dtype=np.int64)

    # pair largest with smallest so each pair's total is near the mean
    desc = np.argsort(-counts, kind="stable")
    pairs = [(int(desc[p]), int(desc[7 - p])) for p in range(4)]
    # pad only to 8 elements (16 B lines) — token-slot compute scales with
    # CA+CB, so every padded slot is wasted matmul columns
    pad8 = lambda n: max(32, -(-int(n) // 8) * 8)
    CA = pad8(counts[desc[0]])
    CB = pad8(counts[desc[4]])

    nc = _CACHE.get((CB, CA))
    if nc is None:
        nc = _build(CB, CA)
        _CACHE[(CB, CA)] = nc

    bf16 = ml_dtypes.bfloat16

    # per-expert gathered activations / coefs (shared by both cores of a pair)
    def gather(e, C):
        ut, cf = tok_lists[e]
        n_e = len(ut)
        xT = np.zeros((H, C), dtype=bf16)
        xT[:, :n_e] = x[ut].T
        xT = xT.reshape(8, P, C).transpose(1, 0, 2)
        coef = np.zeros((C,), dtype=np.float32)
        coef[:n_e] = cf
        return xT, coef

    # per-tile-contiguous flat x layout, matching the kernel's tiling
    tiles_flat = []
    t0 = 0
    for Cseg in (CB, CA):
        for N in _tile_sizes(Cseg):
            tiles_flat.append((t0, N))
            t0 += N

    in_maps = [None] * NCORES
    for pi, (ea, eb) in enumerate(pairs):
        xa, ca_ = gather(ea, CA)
        xb, cb_ = gather(eb, CB)
        # seg 0 = small expert (b), seg 1 = big expert (a)
        xcat = np.concatenate([xb, xa], axis=2)
        xT = np.empty((P, 8 * (CB + CA)), dtype=bf16)
        o = 0
        for (tt0, N) in tiles_flat:
            xT[:, o:o + 8 * N] = xcat[:, :, tt0:tt0 + N].reshape(P, 8 * N)
            o += 8 * N
        coef = np.concatenate([cb_, ca_]).astype(bf16)
        coef = np.ascontiguousarray(
            np.broadcast_to(coef[None, :], (P, CB + CA)))
        for half in range(2):
            sl = slice(half * F2, (half + 1) * F2)
            # w1 blocked [p, seg, fb, ko, j]; w2 blocked [p, seg, mh, kf, j]
            w1h = np.stack([
                w1[e][:, sl].astype(bf16).reshape(8, P, 16, 128)
                .transpose(1, 2, 0, 3)
                for e in (eb, ea)], axis=1)
            w2h = np.stack([
                w2[e][sl, :].astype(bf16).reshape(16, P, 8, 128)
                .transpose(1, 2, 0, 3)
                for e in (eb, ea)], axis=1)
            in_maps[2 * pi + half] = {
                "xT": xT, "w1": np.ascontiguousarray(w1h),
                "w2": np.ascontiguousarray(w2h), "coef": coef,
            }

    res = bass_utils.run_bass_kernel_spmd(nc, in_maps, list(range(NCORES)),
                                          trace=TRACE)
    LAST_RESULTS = res

    # combine: sum the two F-half partials per pair, then scatter-add each
    # expert's rows into the output (token lists are unique per expert)
    out = np.zeros((T, H), dtype=np.float32)
    for pi, (ea, eb) in enumerate(pairs):
        ysum = (res.results[2 * pi]["y"].astype(np.float32)
                + res.results[2 * pi + 1]["y"].astype(np.float32))
        yT = ysum.transpose(1, 0, 2).reshape(H, CB + CA)
        out[tok_lists[eb][0]] += yT[:, :counts[eb]].T
        out[tok_lists[ea][0]] += yT[:, CB:CB + counts[ea]].T
    return out
